# revision 14
# baseline (speedup 1.0000x reference)
"""Trainium2 Bass kernel: batched inverse of homogeneous affine transforms.

Problem: trf (B, 3, 4) fp32 "shift" affines. Padded M = [[I3 + dA, t], [0, 1]].
Output = top 3 rows of M^-1 = [A^-1 | -A^-1 t] where A = I3 + dA.

Closed form via the column-cross-product adjugate:
    Z row r      = cross(a_{r+1}, a_{r+2})   (columns a1,a2,a3, cyclic)
    det          = a1 . Z row 0
    inv          = Z * (1/det)
    col3_r       = sum_j Z[r, j] * (-t_j * rdet)

v9 = the C=1024 low-instruction-count schedule with every SBUF tile kept
under 64KB/partition: the C=1024 attempt with a single 72KB pq tile
produced address-wrap-style corruption (correct at C=512 where the tile is
36KB), so P/Z and Q/W live in separate 36KB tiles (pqP, pqQ) and the
product groups that previously spanned both halves are unmerged (8 product
ops).  C=1024 -> 4 chunks halves the DVE instruction count (~0.45us/op
overhead measured, ~77us/core at C=512).

SBUF plan (48 C-units = 192KB/partition):
  - in/out merged: the output overwrites the input tile (dead after the
    products and tm read it); io pool x2 = 24 units.
  - pqP (P products -> Z) and pqQ (Q products -> det/rdet scratch -> W)
    are DVE-only, so program order replaces double-buffering (x1, 9+9).
  - det partials in pqQ planes 0-2 (dead after zsub), det in plane 3, rdet
    replicated in planes 6-8; the 3x3 scale runs BEFORE W so W's overwrite
    of pqQ lands after the last rdet read.
  - nrt = -t*rdet as 3 single-plane ops all reading rdet plane 6, so DVE
    never waits on the ACT replication (only the scale stage needs it).

Engine assignment per earlier trace analysis: ALL tensor-tensor work on
DVE (DVE+GPSIMD share SBUF bandwidth and GPSIMD is 2.7x less efficient;
DMA-CCE accumulate is broken); ScalarE (independent path) does diag+1,
negt3, rdet replication.
"""

import numpy as np

B = 4_194_304
NCORES = 8
BL = B // NCORES  # 524288 matrices per core
P = 128
C = 1024  # matrices per partition per chunk


def _V(base_ap, off, dims):
    """Strided view of a tile: dims = [(step, count), ...] free dims,
    iterated with the LAST dim innermost. Offset in elements."""
    import concourse.bass as bass

    return bass.AP(
        base_ap.tensor,
        base_ap.offset + off,
        [list(base_ap.ap[0])] + [[int(s), int(n)] for s, n in dims],
    )


# Cross products, P and Q in separate tiles. pqP plane 3r+j = P[r][j],
# pqQ plane 3r+j = Q[r][j]:
#   P[r][j] = A[(j+1)%3][(r+1)%3] * A[(j+2)%3][(r+2)%3]
#   Q[r][j] = A[(j+2)%3][(r+1)%3] * A[(j+1)%3][(r+2)%3]
# (A[i][c] at AoS position 4i+c.)  (dst, out_base_plane, out_dims, l_base,
# l_dims, r_base, r_dims); dims [(step,count),...] with the C-dim appended
# at build.  First four ops cover Z row 0's inputs (planes 0-2 each side).
PROD_GROUPS = [
    # P-A: (r,j) in {0,1}x{0,1} -> planes {0,1,3,4}
    ("P", 0, [(3, 2), (1, 2)], 5, [(1, 2), (4, 2)], 10, [(-2, 2), (-8, 2)]),
    # Q-A: planes {0,1,3,4}
    ("Q", 0, [(3, 2), (1, 2)], 9, [(1, 2), (-8, 2)], 6, [(-2, 2), (4, 2)]),
    # P-B: (r in {0,1}, j=2) -> planes {2,5}
    ("P", 2, [(3, 2)], 1, [(1, 2)], 6, [(-2, 2)]),
    # Q-B: planes {2,5}
    ("Q", 2, [(3, 2)], 5, [(1, 2)], 2, [(-2, 2)]),
    # P-C: (r=2, j in {0,1}) -> planes {6,7}
    ("P", 6, [(1, 2)], 4, [(4, 2)], 9, [(-8, 2)]),
    # Q-C: planes {6,7}
    ("Q", 6, [(1, 2)], 8, [(-8, 2)], 5, [(4, 2)]),
    # P-D: plane 8
    ("P", 8, [], 0, [], 5, []),
    # Q-D: plane 8
    ("Q", 8, [], 4, [], 1, []),
]


def build_nc(bl=BL, c=C):
    import concourse.bass as bass
    import concourse.bacc as bacc
    import concourse.mybir as mybir
    from concourse.tile import TileContext

    f32 = mybir.dt.float32
    nch = bl // (P * c)
    assert bl == nch * P * c

    nc = bacc.Bacc()
    trf = nc.declare_dram_parameter("trf", [bl, 12], f32, isOutput=False)
    out = nc.declare_dram_parameter("out", [bl, 12], f32, isOutput=True)
    trf_t = trf.ap().rearrange("(n p c) m -> n p (c m)", p=P, c=c)
    out_t = out.ap().rearrange("(n p c) m -> n p (c m)", p=P, c=c)

    with TileContext(nc) as tc:
        with (
            tc.tile_pool(name="io", bufs=2) as io,
            tc.tile_pool(name="ng", bufs=2) as ng,
            tc.tile_pool(name="pq", bufs=1) as pqp,
        ):
            for n in range(nch):
                # merged in/out tile: products+tm consume it, then the
                # scale and col3 stages overwrite it with the output
                tio = io.tile([P, 12 * c], f32, tag="tio")
                nc.sync.dma_start(out=tio[:], in_=trf_t[n])

                # diag += 1 in-place: positions {0,5,10} = stride 5 (ACT)
                dg = _V(tio, 0, [(12, c), (5, 3)])
                nc.scalar.add(dg, dg, 1.0)

                # negt3 = -t as 3 contiguous planes (ACT)
                negt3 = ng.tile([P, 3 * c], f32, tag="negt3")
                nc.scalar.mul(
                    _V(negt3, 0, [(c, 3), (1, c)]),
                    _V(tio, 3, [(4, 3), (12, c)]),
                    -1.0,
                )

                # P/Q products: 8 ops on DVE; Z row 0 inputs first
                pqP = pqp.tile([P, 9 * c], f32, tag="pqP")
                pqQ = pqp.tile([P, 9 * c], f32, tag="pqQ")
                dst = {"P": pqP, "Q": pqQ}
                for gi, (dk, ob, od, lb, ld, rb, rd) in enumerate(PROD_GROUPS):
                    nc.vector.tensor_mul(
                        _V(dst[dk], ob * c,
                           [(s * c, k) for s, k in od] + [(1, c)]),
                        _V(tio, lb, ld + [(12, c)]),
                        _V(tio, rb, rd + [(12, c)]),
                    )
                    if gi == 3:
                        # Z row 0 = P - Q
                        nc.vector.tensor_sub(
                            _V(pqP, 0, [(1, 3 * c)]),
                            _V(pqP, 0, [(1, 3 * c)]),
                            _V(pqQ, 0, [(1, 3 * c)]),
                        )
                # Z rows 1-2
                nc.vector.tensor_sub(
                    _V(pqP, 3 * c, [(1, 6 * c)]),
                    _V(pqP, 3 * c, [(1, 6 * c)]),
                    _V(pqQ, 3 * c, [(1, 6 * c)]),
                )

                # det partials -> dead pqQ planes 0-2: tm_i = Z0_i * a1_i
                nc.vector.tensor_mul(
                    _V(pqQ, 0, [(c, 3), (1, c)]),
                    _V(pqP, 0, [(c, 3), (1, c)]),
                    _V(tio, 0, [(4, 3), (12, c)]),
                )
                # det -> pqQ plane 3
                nc.vector.tensor_add(
                    _V(pqQ, 3 * c, [(1, c)]),
                    _V(pqQ, 0, [(1, c)]),
                    _V(pqQ, c, [(1, c)]),
                )
                nc.vector.tensor_add(
                    _V(pqQ, 3 * c, [(1, c)]),
                    _V(pqQ, 3 * c, [(1, c)]),
                    _V(pqQ, 2 * c, [(1, c)]),
                )

                # rdet -> pqQ plane 6 (~51 ULP; det ~ 1); ACT replicates to
                # planes 7,8 for the scale stage (not needed by nrt).
                nc.vector.reciprocal_approx_fast(
                    _V(pqQ, 6 * c, [(1, c)]), _V(pqQ, 3 * c, [(1, c)])
                )
                nc.scalar.copy(
                    _V(pqQ, 7 * c, [(1, c)]), _V(pqQ, 6 * c, [(1, c)])
                )
                nc.scalar.copy(
                    _V(pqQ, 8 * c, [(1, c)]), _V(pqQ, 6 * c, [(1, c)])
                )

                # nrt_j = -t_j * rdet: 3 single-plane ops, all reading
                # plane 6 so DVE never waits on the ACT copies
                for j in range(3):
                    nc.vector.tensor_mul(
                        _V(negt3, j * c, [(1, c)]),
                        _V(negt3, j * c, [(1, c)]),
                        _V(pqQ, 6 * c, [(1, c)]),
                    )

                # inv 3x3: tio[4r+j] = Z[3r+j] * rdet  (before W clobbers
                # the rdet planes)
                for r in range(3):
                    nc.vector.tensor_mul(
                        _V(tio, 4 * r, [(12, c), (1, 3)]),
                        _V(pqP, 3 * r * c, [(1, c), (c, 3)]),
                        _V(pqQ, 6 * c, [(1, c), (c, 3)]),
                    )

                # W[r,j] = Z[3r+j] * nrt_j over all of pqQ
                for r in range(3):
                    nc.vector.tensor_mul(
                        _V(pqQ, 3 * r * c, [(c, 3), (1, c)]),
                        _V(pqP, 3 * r * c, [(c, 3), (1, c)]),
                        _V(negt3, 0, [(c, 3), (1, c)]),
                    )

                # col3: W[.,0] += W[.,1] in place, then + W[.,2] into cols
                nc.vector.tensor_add(
                    _V(pqQ, 0, [(3 * c, 3), (1, c)]),
                    _V(pqQ, 0, [(3 * c, 3), (1, c)]),
                    _V(pqQ, c, [(3 * c, 3), (1, c)]),
                ),
                nc.vector.tensor_add(
                    _V(tio, 3, [(4, 3), (12, c)]),
                    _V(pqQ, 0, [(3 * c, 3), (1, c)]),
                    _V(pqQ, 2 * c, [(3 * c, 3), (1, c)]),
                )

                nc.sync.dma_start(out=out_t[n], in_=tio[:])

    return nc


_CACHE = {}


def _get_nc():
    if "nc" not in _CACHE:
        nc = build_nc()
        nc.finalize()
        _CACHE["nc"] = nc
    return _CACHE["nc"]


def run(trf, trace=False, **spmd_kwargs):
    """Shard, run on 8 cores, gather. Returns (output, BassKernelResults)."""
    from concourse.bass_utils import run_bass_kernel_spmd

    x = np.ascontiguousarray(np.asarray(trf, dtype=np.float32)).reshape(NCORES, BL, 12)
    in_maps = [{"trf": x[i]} for i in range(NCORES)]
    nc = _get_nc()
    res = run_bass_kernel_spmd(
        nc, in_maps, list(range(NCORES)), trace=trace, **spmd_kwargs
    )
    outs = np.stack([np.asarray(res.results[i]["out"]) for i in range(NCORES)])
    return outs.reshape(B, 3, 4).astype(np.float32), res


def kernel(trf):
    return run(trf)[0]


# revision 15
# speedup vs baseline: 1.0389x; 1.0389x over previous
"""Trainium2 Bass kernel: batched inverse of homogeneous affine transforms.

Problem: trf (B, 3, 4) fp32 "shift" affines. Padded M = [[I3 + dA, t], [0, 1]].
Output = top 3 rows of M^-1 = [A^-1 | -A^-1 t] where A = I3 + dA.

Closed form via the column-cross-product adjugate:
    Z row r      = cross(a_{r+1}, a_{r+2})   (columns a1,a2,a3, cyclic)
    det          = a1 . Z row 0
    inv          = Z * (1/det)
    col3_r       = sum_j Z[r, j] * (-t_j * rdet)

Per-core layout: chunks of 128 partitions x C matrices; SBUF input tile is
(128, 12*C), each partition holding C consecutive 12-float AoS matrices.

Schedule (v4b -- best measured of 6 hardware-profiled variants, 372us vs
497us baseline). Trace-established facts that drive the design:
  - DVE and GPSIMD share SBUF port bandwidth: tensor ops on both engines
    concurrently slow each other ~2x and combined throughput equals ONE
    engine's; GPSIMD is also 2.7x less efficient per element. So ALL
    tensor-tensor work runs on DVE and GPSIMD runs nothing.
  - ScalarE has an independent SBUF path (its op times are unchanged under
    full DVE load), so it does the 1-input work: diag+1, negt3 = -t, rdet
    replication.
  - The DMA CCE accumulate path (SBUF->SBUF accum-add) does not actually
    read-modify-write here (tried: overwrites), and CCE has no subtract,
    so Z = P - Q stays on DVE.
  - 18 cross products emitted as 6 grouped DVE ops (affine sub-grids of
    the (r,j) cofactor grid; the P/Q "B" and "D" sub-grids merge via a 4th
    AP dim), amortizing the ~0.5-0.7us per-op DVE overhead; the row-0
    planes are produced first so the det chain starts early.
  - W = Z * nrt with nrt = -t * rdet folded once: all W operands are
    plane-contiguous (strided DVE reads cost +0.6ns/elem; writes cheap).
  - det partials are staged in the s tile (the det adds precede s1's
    overwrite on the same engine, so program order protects them).
"""

import numpy as np

B = 4_194_304
NCORES = 8
BL = B // NCORES  # 524288 matrices per core
P = 128
C = 512  # matrices per partition per chunk


def _V(base_ap, off, dims):
    """Strided view of a tile: dims = [(step, count), ...] free dims,
    iterated with the LAST dim innermost. Offset in elements."""
    import concourse.bass as bass

    return bass.AP(
        base_ap.tensor,
        base_ap.offset + off,
        [list(base_ap.ap[0])] + [[int(s), int(n)] for s, n in dims],
    )


# Grouped cross products. pq plane 3r+j = P[r][j], 9+3r+j = Q[r][j]:
#   P[r][j] = A[(j+1)%3][(r+1)%3] * A[(j+2)%3][(r+2)%3]
#   Q[r][j] = A[(j+2)%3][(r+1)%3] * A[(j+1)%3][(r+2)%3]
# (A[i][c] at AoS position 4i+c.)  (out_base_plane, out_dims, l_base,
# l_dims, r_base, r_dims); dims [(step,count),...], C-dim appended at
# build.  First three ops cover planes {0,1,2, 9,10,11} = Z row 0.
PROD_GROUPS = [
    # P-A: (r,j) in {0,1}x{0,1} -> planes {0,1,3,4}
    (0, [(3, 2), (1, 2)], 5, [(1, 2), (4, 2)], 10, [(-2, 2), (-8, 2)]),
    # Q-A: planes {9,10,12,13}
    (9, [(3, 2), (1, 2)], 9, [(1, 2), (-8, 2)], 6, [(-2, 2), (4, 2)]),
    # P-B + Q-B merged over q: planes {2,5} u {11,14}
    (2, [(9, 2), (3, 2)], 1, [(4, 2), (1, 2)], 6, [(-4, 2), (-2, 2)]),
    # P-C: planes {6,7}
    (6, [(1, 2)], 4, [(4, 2)], 9, [(-8, 2)]),
    # Q-C: planes {15,16}
    (15, [(1, 2)], 8, [(-8, 2)], 5, [(4, 2)]),
    # P-D + Q-D merged: planes {8, 17}
    (8, [(9, 2)], 0, [(4, 2)], 5, [(-4, 2)]),
]


def build_nc(bl=BL, c=C):
    import concourse.bass as bass
    import concourse.bacc as bacc
    import concourse.mybir as mybir
    from concourse.tile import TileContext

    f32 = mybir.dt.float32
    nch = bl // (P * c)
    assert bl == nch * P * c

    nc = bacc.Bacc()
    trf = nc.declare_dram_parameter("trf", [bl, 12], f32, isOutput=False)
    out = nc.declare_dram_parameter("out", [bl, 12], f32, isOutput=True)
    trf_t = trf.ap().rearrange("(n p c) m -> n p (c m)", p=P, c=c)
    out_t = out.ap().rearrange("(n p c) m -> n p (c m)", p=P, c=c)

    with TileContext(nc) as tc:
        with (
            tc.tile_pool(name="io", bufs=2) as io,
            tc.tile_pool(name="tmp", bufs=2) as tmp,
            tc.tile_pool(name="det", bufs=1) as dpool,
        ):
            for n in range(nch):
                tin = io.tile([P, 12 * c], f32, tag="tin")
                nc.sync.dma_start(out=tin[:], in_=trf_t[n])

                # diag += 1 in-place: positions {0,5,10} = stride 5 (ACT)
                dg = _V(tin, 0, [(12, c), (5, 3)])
                nc.scalar.add(dg, dg, 1.0)

                # negt3 = -t as 3 contiguous planes (ACT)
                negt3 = tmp.tile([P, 3 * c], f32, tag="negt3")
                nc.scalar.mul(
                    _V(negt3, 0, [(c, 3), (1, c)]),
                    _V(tin, 3, [(4, 3), (12, c)]),
                    -1.0,
                )

                # P/Q products: 6 grouped ops on DVE; Z row 0 inputs first
                pq = tmp.tile([P, 18 * c], f32, tag="pq")
                for gi, (ob, od, lb, ld, rb, rd) in enumerate(PROD_GROUPS):
                    nc.vector.tensor_mul(
                        _V(pq, ob * c, [(s * c, k) for s, k in od] + [(1, c)]),
                        _V(tin, lb, ld + [(12, c)]),
                        _V(tin, rb, rd + [(12, c)]),
                    )
                    if gi == 2:
                        # Z row 0 = P - Q
                        nc.vector.tensor_sub(
                            _V(pq, 0, [(1, 3 * c)]),
                            _V(pq, 0, [(1, 3 * c)]),
                            _V(pq, 9 * c, [(1, 3 * c)]),
                        )
                # Z rows 1-2
                nc.vector.tensor_sub(
                    _V(pq, 3 * c, [(1, 6 * c)]),
                    _V(pq, 3 * c, [(1, 6 * c)]),
                    _V(pq, 12 * c, [(1, 6 * c)]),
                )

                # det = a1 . Z row 0; partials staged in the s tile
                s_tm = dpool.tile([P, 3 * c], f32, tag="s_tm")
                nc.vector.tensor_mul(
                    _V(s_tm, 0, [(c, 3), (1, c)]),
                    _V(pq, 0, [(c, 3), (1, c)]),
                    _V(tin, 0, [(4, 3), (12, c)]),
                )
                det = dpool.tile([P, c], f32, tag="det")
                nc.vector.tensor_add(
                    det[:], _V(s_tm, 0, [(1, c)]), _V(s_tm, c, [(1, c)])
                )
                nc.vector.tensor_add(det[:], det[:], _V(s_tm, 2 * c, [(1, c)]))

                # rdet = 1/det (~51 ULP, det ~ 1) -> rdet3 plane 0; ACT
                # replicates to planes 1,2.
                rdet3 = dpool.tile([P, 3 * c], f32, tag="rdet3")
                nc.vector.reciprocal_approx_fast(_V(rdet3, 0, [(1, c)]), det[:])
                nc.scalar.copy(_V(rdet3, c, [(1, c)]), _V(rdet3, 0, [(1, c)]))
                nc.scalar.copy(_V(rdet3, 2 * c, [(1, c)]), _V(rdet3, 0, [(1, c)]))

                # nrt_j = -t_j * rdet (in place over negt3)
                nc.vector.tensor_mul(
                    _V(negt3, 0, [(c, 3), (1, c)]),
                    _V(negt3, 0, [(c, 3), (1, c)]),
                    _V(rdet3, 0, [(c, 3), (1, c)]),
                )

                # W[r,j] = Z[3r+j] * nrt_j into dead Q planes
                for r in range(3):
                    nc.vector.tensor_mul(
                        _V(pq, (9 + 3 * r) * c, [(c, 3), (1, c)]),
                        _V(pq, 3 * r * c, [(c, 3), (1, c)]),
                        _V(negt3, 0, [(c, 3), (1, c)]),
                    )

                # col3: s = W[r,0]+W[r,1] (planes), then into tout cols
                tout = io.tile([P, 12 * c], f32, tag="tout")
                nc.vector.tensor_add(
                    _V(s_tm, 0, [(c, 3), (1, c)]),
                    _V(pq, 9 * c, [(3 * c, 3), (1, c)]),
                    _V(pq, 10 * c, [(3 * c, 3), (1, c)]),
                )
                nc.vector.tensor_add(
                    _V(tout, 3, [(4, 3), (12, c)]),
                    _V(s_tm, 0, [(c, 3), (1, c)]),
                    _V(pq, 11 * c, [(3 * c, 3), (1, c)]),
                )

                # inv 3x3: tout[4r+j] = Z[3r+j] * rdet (3 ops, DVE)
                for r in range(3):
                    nc.vector.tensor_mul(
                        _V(tout, 4 * r, [(12, c), (1, 3)]),
                        _V(pq, 3 * r * c, [(1, c), (c, 3)]),
                        _V(rdet3, 0, [(1, c), (c, 3)]),
                    )

                nc.sync.dma_start(out=out_t[n], in_=tout[:])

    return nc


_CACHE = {}


def _get_nc():
    if "nc" not in _CACHE:
        nc = build_nc()
        nc.finalize()
        _CACHE["nc"] = nc
    return _CACHE["nc"]


def run(trf, trace=False, **spmd_kwargs):
    """Shard, run on 8 cores, gather. Returns (output, BassKernelResults)."""
    from concourse.bass_utils import run_bass_kernel_spmd

    x = np.ascontiguousarray(np.asarray(trf, dtype=np.float32)).reshape(NCORES, BL, 12)
    in_maps = [{"trf": x[i]} for i in range(NCORES)]
    nc = _get_nc()
    res = run_bass_kernel_spmd(
        nc, in_maps, list(range(NCORES)), trace=trace, **spmd_kwargs
    )
    outs = np.stack([np.asarray(res.results[i]["out"]) for i in range(NCORES)])
    return outs.reshape(B, 3, 4).astype(np.float32), res


def kernel(trf):
    return run(trf)[0]


# revision 16
# speedup vs baseline: 1.0426x; 1.0035x over previous
"""Trainium2 Bass kernel: batched inverse of homogeneous affine transforms.

Problem: trf (B, 3, 4) fp32 "shift" affines. Padded M = [[I3 + dA, t], [0, 1]].
Output = top 3 rows of M^-1 = [A^-1 | -A^-1 t] where A = I3 + dA.

Closed form via the column-cross-product adjugate:
    Z row r      = cross(a_{r+1}, a_{r+2})   (columns a1,a2,a3, cyclic)
    det          = a1 . Z row 0
    inv          = Z * (1/det)
    col3_r       = sum_j Z[r, j] * (-t_j * rdet)

Per-core layout: chunks of 128 partitions x C matrices; SBUF input tile is
(128, 12*C), each partition holding C consecutive 12-float AoS matrices.

Schedule (v4b -- best measured of 6 hardware-profiled variants, 372us vs
497us baseline). Trace-established facts that drive the design:
  - DVE and GPSIMD share SBUF port bandwidth: tensor ops on both engines
    concurrently slow each other ~2x and combined throughput equals ONE
    engine's; GPSIMD is also 2.7x less efficient per element. So ALL
    tensor-tensor work runs on DVE and GPSIMD runs nothing.
  - ScalarE has an independent SBUF path (its op times are unchanged under
    full DVE load), so it does the 1-input work: diag+1, negt3 = -t, rdet
    replication.
  - The DMA CCE accumulate path (SBUF->SBUF accum-add) does not actually
    read-modify-write here (tried: overwrites), and CCE has no subtract,
    so Z = P - Q stays on DVE.
  - 18 cross products emitted as 6 grouped DVE ops (affine sub-grids of
    the (r,j) cofactor grid; the P/Q "B" and "D" sub-grids merge via a 4th
    AP dim), amortizing the ~0.5-0.7us per-op DVE overhead; the row-0
    planes are produced first so the det chain starts early.
  - W = Z * nrt with nrt = -t * rdet folded once: all W operands are
    plane-contiguous (strided DVE reads cost +0.6ns/elem; writes cheap).
  - det partials are staged in the s tile (the det adds precede s1's
    overwrite on the same engine, so program order protects them).
"""

import numpy as np

B = 4_194_304
NCORES = 8
BL = B // NCORES  # 524288 matrices per core
P = 128
C = 512  # matrices per partition per chunk


def _V(base_ap, off, dims):
    """Strided view of a tile: dims = [(step, count), ...] free dims,
    iterated with the LAST dim innermost. Offset in elements."""
    import concourse.bass as bass

    return bass.AP(
        base_ap.tensor,
        base_ap.offset + off,
        [list(base_ap.ap[0])] + [[int(s), int(n)] for s, n in dims],
    )


# Grouped cross products. pq plane 3r+j = P[r][j], 9+3r+j = Q[r][j]:
#   P[r][j] = A[(j+1)%3][(r+1)%3] * A[(j+2)%3][(r+2)%3]
#   Q[r][j] = A[(j+2)%3][(r+1)%3] * A[(j+1)%3][(r+2)%3]
# (A[i][c] at AoS position 4i+c.)  (out_base_plane, out_dims, l_base,
# l_dims, r_base, r_dims); dims [(step,count),...], C-dim appended at
# build.  First three ops cover planes {0,1,2, 9,10,11} = Z row 0.
PROD_GROUPS = [
    # P-A: (r,j) in {0,1}x{0,1} -> planes {0,1,3,4}
    (0, [(3, 2), (1, 2)], 5, [(1, 2), (4, 2)], 10, [(-2, 2), (-8, 2)]),
    # Q-A: planes {9,10,12,13}
    (9, [(3, 2), (1, 2)], 9, [(1, 2), (-8, 2)], 6, [(-2, 2), (4, 2)]),
    # P-B + Q-B merged over q: planes {2,5} u {11,14}
    (2, [(9, 2), (3, 2)], 1, [(4, 2), (1, 2)], 6, [(-4, 2), (-2, 2)]),
    # P-C: planes {6,7}
    (6, [(1, 2)], 4, [(4, 2)], 9, [(-8, 2)]),
    # Q-C: planes {15,16}
    (15, [(1, 2)], 8, [(-8, 2)], 5, [(4, 2)]),
    # P-D + Q-D merged: planes {8, 17}
    (8, [(9, 2)], 0, [(4, 2)], 5, [(-4, 2)]),
]


def build_nc(bl=BL, c=C):
    import concourse.bass as bass
    import concourse.bacc as bacc
    import concourse.mybir as mybir
    from concourse.tile import TileContext

    f32 = mybir.dt.float32
    nch = bl // (P * c)
    assert bl == nch * P * c

    nc = bacc.Bacc()
    trf = nc.declare_dram_parameter("trf", [bl, 12], f32, isOutput=False)
    out = nc.declare_dram_parameter("out", [bl, 12], f32, isOutput=True)
    trf_t = trf.ap().rearrange("(n p c) m -> n p (c m)", p=P, c=c)
    out_t = out.ap().rearrange("(n p c) m -> n p (c m)", p=P, c=c)

    with TileContext(nc) as tc:
        with (
            tc.tile_pool(name="io", bufs=2) as io,
            tc.tile_pool(name="tmp", bufs=2) as tmp,
            tc.tile_pool(name="det", bufs=1) as dpool,
        ):
            for n in range(nch):
                tin = io.tile([P, 12 * c], f32, tag="tin")
                nc.sync.dma_start(out=tin[:], in_=trf_t[n])

                # diag += 1 in-place: positions {0,5,10} = stride 5 (ACT)
                dg = _V(tin, 0, [(12, c), (5, 3)])
                nc.scalar.add(dg, dg, 1.0)

                # negt3 = -t as 3 contiguous planes (ACT)
                pq = tmp.tile([P, 18 * c], f32, tag="pq")
                negt3 = tmp.tile([P, 3 * c], f32, tag="negt3")
                nc.scalar.mul(
                    _V(negt3, 0, [(c, 3), (1, c)]),
                    _V(tin, 3, [(4, 3), (12, c)]),
                    -1.0,
                )

                # P/Q products: 6 grouped ops on DVE; Z row 0 inputs first
                for gi, (ob, od, lb, ld, rb, rd) in enumerate(PROD_GROUPS):
                    nc.vector.tensor_mul(
                        _V(pq, ob * c, [(s * c, k) for s, k in od] + [(1, c)]),
                        _V(tin, lb, ld + [(12, c)]),
                        _V(tin, rb, rd + [(12, c)]),
                    )
                    if gi == 2:
                        # Z row 0 = P - Q
                        nc.vector.tensor_sub(
                            _V(pq, 0, [(1, 3 * c)]),
                            _V(pq, 0, [(1, 3 * c)]),
                            _V(pq, 9 * c, [(1, 3 * c)]),
                        )
                # Z rows 1-2
                nc.vector.tensor_sub(
                    _V(pq, 3 * c, [(1, 6 * c)]),
                    _V(pq, 3 * c, [(1, 6 * c)]),
                    _V(pq, 12 * c, [(1, 6 * c)]),
                )

                # det = a1 . Z row 0; partials staged in the s tile
                rdet3 = dpool.tile([P, 3 * c], f32, tag="rdet3")
                s_tm = dpool.tile([P, 3 * c], f32, tag="s_tm")
                nc.vector.tensor_mul(
                    _V(s_tm, 0, [(c, 3), (1, c)]),
                    _V(pq, 0, [(c, 3), (1, c)]),
                    _V(tin, 0, [(4, 3), (12, c)]),
                )
                det = dpool.tile([P, c], f32, tag="det")
                nc.vector.tensor_add(
                    det[:], _V(s_tm, 0, [(1, c)]), _V(s_tm, c, [(1, c)])
                )
                nc.vector.tensor_add(det[:], det[:], _V(s_tm, 2 * c, [(1, c)]))

                # rdet = 1/det (~51 ULP, det ~ 1) -> rdet3 plane 0; ACT
                # replicates to planes 1,2.
                nc.vector.reciprocal_approx_fast(_V(rdet3, 0, [(1, c)]), det[:])
                nc.scalar.copy(_V(rdet3, c, [(1, c)]), _V(rdet3, 0, [(1, c)]))
                nc.scalar.copy(_V(rdet3, 2 * c, [(1, c)]), _V(rdet3, 0, [(1, c)]))

                # nrt_j = -t_j * rdet (in place over negt3)
                nc.vector.tensor_mul(
                    _V(negt3, 0, [(c, 3), (1, c)]),
                    _V(negt3, 0, [(c, 3), (1, c)]),
                    _V(rdet3, 0, [(c, 3), (1, c)]),
                )

                # W[r,j] = Z[3r+j] * nrt_j into dead Q planes
                for r in range(3):
                    nc.vector.tensor_mul(
                        _V(pq, (9 + 3 * r) * c, [(c, 3), (1, c)]),
                        _V(pq, 3 * r * c, [(c, 3), (1, c)]),
                        _V(negt3, 0, [(c, 3), (1, c)]),
                    )

                # col3: s = W[r,0]+W[r,1] (planes), then into tout cols
                tout = io.tile([P, 12 * c], f32, tag="tout")
                nc.vector.tensor_add(
                    _V(s_tm, 0, [(c, 3), (1, c)]),
                    _V(pq, 9 * c, [(3 * c, 3), (1, c)]),
                    _V(pq, 10 * c, [(3 * c, 3), (1, c)]),
                )
                nc.vector.tensor_add(
                    _V(tout, 3, [(4, 3), (12, c)]),
                    _V(s_tm, 0, [(c, 3), (1, c)]),
                    _V(pq, 11 * c, [(3 * c, 3), (1, c)]),
                )

                # inv 3x3: tout[4r+j] = Z[3r+j] * rdet (3 ops, DVE)
                for r in range(3):
                    nc.vector.tensor_mul(
                        _V(tout, 4 * r, [(12, c), (1, 3)]),
                        _V(pq, 3 * r * c, [(1, c), (c, 3)]),
                        _V(rdet3, 0, [(1, c), (c, 3)]),
                    )

                nc.sync.dma_start(out=out_t[n], in_=tout[:])

    return nc


_CACHE = {}


def _get_nc():
    if "nc" not in _CACHE:
        nc = build_nc()
        nc.finalize()
        _CACHE["nc"] = nc
    return _CACHE["nc"]


def run(trf, trace=False, **spmd_kwargs):
    """Shard, run on 8 cores, gather. Returns (output, BassKernelResults)."""
    from concourse.bass_utils import run_bass_kernel_spmd

    x = np.ascontiguousarray(np.asarray(trf, dtype=np.float32)).reshape(NCORES, BL, 12)
    in_maps = [{"trf": x[i]} for i in range(NCORES)]
    nc = _get_nc()
    res = run_bass_kernel_spmd(
        nc, in_maps, list(range(NCORES)), trace=trace, **spmd_kwargs
    )
    outs = np.stack([np.asarray(res.results[i]["out"]) for i in range(NCORES)])
    return outs.reshape(B, 3, 4).astype(np.float32), res


def kernel(trf):
    return run(trf)[0]


# revision 17
# speedup vs baseline: 1.0683x; 1.0247x over previous
"""Trainium2 Bass kernel: batched inverse of homogeneous affine transforms.

Problem: trf (B, 3, 4) fp32 "shift" affines. Padded M = [[I3 + dA, t], [0, 1]].
Output = top 3 rows of M^-1 = [A^-1 | -A^-1 t] where A = I3 + dA.

Closed form via the column-cross-product adjugate:
    Z row r      = cross(a_{r+1}, a_{r+2})   (columns a1,a2,a3, cyclic)
    det          = a1 . Z row 0
    inv          = Z * (1/det)
    col3_r       = sum_j Z[r, j] * (-t_j * rdet)

Per-core layout: chunks of 128 partitions x C matrices; SBUF input tile is
(128, 12*C), each partition holding C consecutive 12-float AoS matrices.

Schedule (v4b -- best measured of 6 hardware-profiled variants, 372us vs
497us baseline). Trace-established facts that drive the design:
  - DVE and GPSIMD share SBUF port bandwidth: tensor ops on both engines
    concurrently slow each other ~2x and combined throughput equals ONE
    engine's; GPSIMD is also 2.7x less efficient per element. So ALL
    tensor-tensor work runs on DVE and GPSIMD runs nothing.
  - ScalarE has an independent SBUF path (its op times are unchanged under
    full DVE load), so it does the 1-input work: diag+1, negt3 = -t, rdet
    replication.
  - The DMA CCE accumulate path (SBUF->SBUF accum-add) does not actually
    read-modify-write here (tried: overwrites), and CCE has no subtract,
    so Z = P - Q stays on DVE.
  - 18 cross products emitted as 6 grouped DVE ops (affine sub-grids of
    the (r,j) cofactor grid; the P/Q "B" and "D" sub-grids merge via a 4th
    AP dim), amortizing the ~0.5-0.7us per-op DVE overhead; the row-0
    planes are produced first so the det chain starts early.
  - W = Z * nrt with nrt = -t * rdet folded once: all W operands are
    plane-contiguous (strided DVE reads cost +0.6ns/elem; writes cheap).
  - det partials are staged in the s tile (the det adds precede s1's
    overwrite on the same engine, so program order protects them).
"""

import numpy as np

B = 4_194_304
NCORES = 8
BL = B // NCORES  # 524288 matrices per core
P = 128
C = 512  # matrices per partition per chunk


def _V(base_ap, off, dims):
    """Strided view of a tile: dims = [(step, count), ...] free dims,
    iterated with the LAST dim innermost. Offset in elements."""
    import concourse.bass as bass

    return bass.AP(
        base_ap.tensor,
        base_ap.offset + off,
        [list(base_ap.ap[0])] + [[int(s), int(n)] for s, n in dims],
    )


# Grouped cross products. pq plane 3r+j = P[r][j], 9+3r+j = Q[r][j]:
#   P[r][j] = A[(j+1)%3][(r+1)%3] * A[(j+2)%3][(r+2)%3]
#   Q[r][j] = A[(j+2)%3][(r+1)%3] * A[(j+1)%3][(r+2)%3]
# (A[i][c] at AoS position 4i+c.)  (out_base_plane, out_dims, l_base,
# l_dims, r_base, r_dims); dims [(step,count),...], C-dim appended at
# build.  First three ops cover planes {0,1,2, 9,10,11} = Z row 0.
PROD_GROUPS = [
    # P-A: (r,j) in {0,1}x{0,1} -> planes {0,1,3,4}
    (0, [(3, 2), (1, 2)], 5, [(1, 2), (4, 2)], 10, [(-2, 2), (-8, 2)]),
    # Q-A: planes {9,10,12,13}
    (9, [(3, 2), (1, 2)], 9, [(1, 2), (-8, 2)], 6, [(-2, 2), (4, 2)]),
    # P-B + Q-B merged over q: planes {2,5} u {11,14}
    (2, [(9, 2), (3, 2)], 1, [(4, 2), (1, 2)], 6, [(-4, 2), (-2, 2)]),
    # P-C: planes {6,7}
    (6, [(1, 2)], 4, [(4, 2)], 9, [(-8, 2)]),
    # Q-C: planes {15,16}
    (15, [(1, 2)], 8, [(-8, 2)], 5, [(4, 2)]),
    # P-D + Q-D merged: planes {8, 17}
    (8, [(9, 2)], 0, [(4, 2)], 5, [(-4, 2)]),
]


def build_nc(bl=BL, c=C):
    import concourse.bass as bass
    import concourse.bacc as bacc
    import concourse.mybir as mybir
    from concourse.tile import TileContext

    f32 = mybir.dt.float32
    nch = bl // (P * c)
    assert bl == nch * P * c

    nc = bacc.Bacc()
    trf = nc.declare_dram_parameter("trf", [bl, 12], f32, isOutput=False)
    out = nc.declare_dram_parameter("out", [bl, 12], f32, isOutput=True)
    trf_t = trf.ap().rearrange("(n p c) m -> n p (c m)", p=P, c=c)
    out_t = out.ap().rearrange("(n p c) m -> n p (c m)", p=P, c=c)

    with TileContext(nc) as tc:
        with (
            tc.tile_pool(name="io", bufs=2) as io,
            tc.tile_pool(name="tmp", bufs=2) as tmp,
            tc.tile_pool(name="det", bufs=1) as dpool,
        ):
            for n in range(nch):
                tin = io.tile([P, 12 * c], f32, tag="tin")
                nc.sync.dma_start(out=tin[:], in_=trf_t[n])

                # diag += 1 in-place: positions {0,5,10} = stride 5 (ACT)
                dg = _V(tin, 0, [(12, c), (5, 3)])
                nc.scalar.add(dg, dg, 1.0)

                # negt3 = -t as 3 contiguous planes (ACT)
                pq = tmp.tile([P, 18 * c], f32, tag="pq")
                negt3 = tmp.tile([P, 3 * c], f32, tag="negt3")
                nc.scalar.mul(
                    _V(negt3, 0, [(c, 3), (1, c)]),
                    _V(tin, 3, [(4, 3), (12, c)]),
                    -1.0,
                )

                # a1g = column 0 gathered contiguous (ACT) so tm's
                # second operand is not a 48B-strided DVE read
                a1g = tmp.tile([P, 3 * c], f32, tag="a1g")
                nc.scalar.copy(
                    _V(a1g, 0, [(c, 3), (1, c)]),
                    _V(tin, 0, [(4, 3), (12, c)]),
                )

                # P/Q products: 6 grouped ops on DVE; Z row 0 inputs first
                for gi, (ob, od, lb, ld, rb, rd) in enumerate(PROD_GROUPS):
                    nc.vector.tensor_mul(
                        _V(pq, ob * c, [(s * c, k) for s, k in od] + [(1, c)]),
                        _V(tin, lb, ld + [(12, c)]),
                        _V(tin, rb, rd + [(12, c)]),
                    )
                    if gi == 2:
                        # Z row 0 = P - Q
                        nc.vector.tensor_sub(
                            _V(pq, 0, [(1, 3 * c)]),
                            _V(pq, 0, [(1, 3 * c)]),
                            _V(pq, 9 * c, [(1, 3 * c)]),
                        )
                # Z rows 1-2
                nc.vector.tensor_sub(
                    _V(pq, 3 * c, [(1, 6 * c)]),
                    _V(pq, 3 * c, [(1, 6 * c)]),
                    _V(pq, 12 * c, [(1, 6 * c)]),
                )

                # det = a1 . Z row 0; partials staged in the s tile
                rdet3 = dpool.tile([P, 3 * c], f32, tag="rdet3")
                s_tm = dpool.tile([P, 3 * c], f32, tag="s_tm")
                nc.vector.tensor_mul(
                    _V(s_tm, 0, [(c, 3), (1, c)]),
                    _V(pq, 0, [(c, 3), (1, c)]),
                    _V(a1g, 0, [(c, 3), (1, c)]),
                )
                det = dpool.tile([P, c], f32, tag="det")
                nc.vector.tensor_add(
                    det[:], _V(s_tm, 0, [(1, c)]), _V(s_tm, c, [(1, c)])
                )
                nc.vector.tensor_add(det[:], det[:], _V(s_tm, 2 * c, [(1, c)]))

                # rdet = 1/det (~51 ULP, det ~ 1) -> rdet3 plane 0; ACT
                # replicates to planes 1,2.
                nc.vector.reciprocal_approx_fast(_V(rdet3, 0, [(1, c)]), det[:])
                nc.scalar.copy(_V(rdet3, c, [(1, c)]), _V(rdet3, 0, [(1, c)]))
                nc.scalar.copy(_V(rdet3, 2 * c, [(1, c)]), _V(rdet3, 0, [(1, c)]))

                # nrt_j = -t_j * rdet (in place over negt3)
                nc.vector.tensor_mul(
                    _V(negt3, 0, [(c, 3), (1, c)]),
                    _V(negt3, 0, [(c, 3), (1, c)]),
                    _V(rdet3, 0, [(c, 3), (1, c)]),
                )

                # W[r,j] = Z[3r+j] * nrt_j into dead Q planes
                for r in range(3):
                    nc.vector.tensor_mul(
                        _V(pq, (9 + 3 * r) * c, [(c, 3), (1, c)]),
                        _V(pq, 3 * r * c, [(c, 3), (1, c)]),
                        _V(negt3, 0, [(c, 3), (1, c)]),
                    )

                # col3: s = W[r,0]+W[r,1] (planes), then into tout cols
                tout = io.tile([P, 12 * c], f32, tag="tout")
                nc.vector.tensor_add(
                    _V(s_tm, 0, [(c, 3), (1, c)]),
                    _V(pq, 9 * c, [(3 * c, 3), (1, c)]),
                    _V(pq, 10 * c, [(3 * c, 3), (1, c)]),
                )
                nc.vector.tensor_add(
                    _V(tout, 3, [(4, 3), (12, c)]),
                    _V(s_tm, 0, [(c, 3), (1, c)]),
                    _V(pq, 11 * c, [(3 * c, 3), (1, c)]),
                )

                # inv 3x3: tout[4r+j] = Z[3r+j] * rdet (3 ops, DVE)
                for r in range(3):
                    nc.vector.tensor_mul(
                        _V(tout, 4 * r, [(12, c), (1, 3)]),
                        _V(pq, 3 * r * c, [(1, c), (c, 3)]),
                        _V(rdet3, 0, [(1, c), (c, 3)]),
                    )

                nc.sync.dma_start(out=out_t[n], in_=tout[:])

    return nc


_CACHE = {}


def _get_nc():
    if "nc" not in _CACHE:
        nc = build_nc()
        nc.finalize()
        _CACHE["nc"] = nc
    return _CACHE["nc"]


def run(trf, trace=False, **spmd_kwargs):
    """Shard, run on 8 cores, gather. Returns (output, BassKernelResults)."""
    from concourse.bass_utils import run_bass_kernel_spmd

    x = np.ascontiguousarray(np.asarray(trf, dtype=np.float32)).reshape(NCORES, BL, 12)
    in_maps = [{"trf": x[i]} for i in range(NCORES)]
    nc = _get_nc()
    res = run_bass_kernel_spmd(
        nc, in_maps, list(range(NCORES)), trace=trace, **spmd_kwargs
    )
    outs = np.stack([np.asarray(res.results[i]["out"]) for i in range(NCORES)])
    return outs.reshape(B, 3, 4).astype(np.float32), res


def kernel(trf):
    return run(trf)[0]
